# revision 38
# baseline (speedup 1.0000x reference)
"""GRU (B=64, T=512, DIN=D=512) on 8 Trainium2 NeuronCores.

Strategy
--------
Data-parallel over batch: each core owns BL = 8 batch rows, weights are
replicated (per the sharding hint).  Per core:

1. Projection phase: xg = X @ W_g + b_g for g in {z, r, h} as bf16 GEMMs
   (X, W pre-converted to bf16 on the host) with W stationary and X^T
   streaming, written into an SBUF-resident pre-activation buffer
   xall[p, g, m, t*BL+b] (bf16) by ScalarE Identity-with-bias ops.
   Chunk 0 runs as a prologue; chunks 1-2 interleave into scan chunk 0,
   chunk c+2 into scan chunk c after that, so projection time is almost
   entirely hidden in the scan's PE idle windows.

2. Scan phase (the sequential part): state is kept transposed,
   hT [128 partitions = d%128, KT=4 k-tiles, BL=8], so that
   - the recurrent matmuls are psum[m] += U[k,m].T @ hmT[k] (U stationary,
     state streaming, output already transposed), and
   - all elementwise work runs on fat [128, *] tiles.
   The x-projection term is accumulated into PSUM by an identity matmul
   (start=True) so the activations read PSUM directly.

   The per-step serial chain is pipelined at m-half / k-half granularity:
   - r-gate MMs are ordered so the m01 half of the r pre-activation
     finishes 8 MMs early; sigmoid(r) and r*hm run in m-halves, letting
     the h-gate's k01 matmuls start while the m23 half is still in the
     ACT/DVE pipe.
   - the h-gate runs in two k-waves (k01 after rhm[m01], k23 after
     rhm[m23]); wave 2 touches the m01 output columns first so tanh on
     the first half starts 4 MMs early.
   - the update gate is computed as zc = sigmoid(-zpre) = 1 - z and the
     blend refactored as h = zc*hh - c1n with c1n = (zc-1)*hm computed
     off the critical path in ONE fused DVE op (scalar_tensor_tensor).
   - blend runs in k-halves so the next step's k01 matmuls start after
     half the blend.

The mask input: reference semantics are h_t = z*(m_{t-1}*h_{t-1}) + ...,
i.e. the *shifted* mask multiplies the previous state.  For the all-ones
mask (what setup_inputs produces) this is the identity, so the fast path
skips the multiply; a general path (host-broadcast shifted mask streamed
from DRAM, one extra DVE mul per step) handles arbitrary 0/1 masks.
"""

import numpy as np
from contextlib import ExitStack

import concourse.bass as bass
import concourse.bacc as bacc
import concourse.mybir as mybir
import concourse.tile as tile
from concourse.tile import add_dep_helper
from concourse.bass_utils import run_bass_kernel_spmd

FP32 = mybir.dt.float32
BF16 = mybir.dt.bfloat16
AF = mybir.ActivationFunctionType
ALU = mybir.AluOpType

B, T, DIN, D = 64, 512, 512, 512
NCORES = 8
BL = B // NCORES            # 8 batch rows per core
KT = DIN // 128             # 4 contraction tiles
MT = D // 128               # 4 output tiles
P = 128


def build_nc(T_=T, masked=False):
    """Build the single-core SPMD program (identical on all 8 cores)."""
    tl = min(64, T_)                     # steps per chunk
    sch = T_ // tl                       # chunks
    pcw = tl * BL                        # chunk width in columns (512)

    nc = bacc.Bacc(None, target_bir_lowering=False, debug=False)

    xT = nc.dram_tensor("xT", [DIN, T_ * BL], BF16, kind="ExternalInput")
    w_lay = {g: nc.dram_tensor(f"W{g}", [P, KT * D], BF16, kind="ExternalInput")
             for g in "zrh"}
    u_lay = {g: nc.dram_tensor(f"U{g}", [P, KT * D], BF16, kind="ExternalInput")
             for g in "zrh"}
    u_lay["rn"] = nc.dram_tensor("Urn", [P, KT * D], BF16,
                                 kind="ExternalInput")
    b4 = {g: nc.dram_tensor(f"b{g}", [P, MT], FP32, kind="ExternalInput")
          for g in "zrh"}
    eye_d = nc.dram_tensor("eye", [P, P], BF16, kind="ExternalInput")
    mb = None
    if masked:
        mb = nc.dram_tensor("mb", [T_, P, KT * BL], FP32, kind="ExternalInput")
    hT_out = nc.dram_tensor("hT_out", [D, BL], FP32, kind="ExternalOutput")

    with tile.TileContext(nc) as tc, ExitStack() as ctx:
        upool = ctx.enter_context(tc.tile_pool(name="upool", bufs=1))
        wpool = ctx.enter_context(tc.tile_pool(name="wpool", bufs=1))
        bp = ctx.enter_context(tc.tile_pool(name="bp", bufs=1))
        xap = ctx.enter_context(tc.tile_pool(name="xap", bufs=1))
        xtp = ctx.enter_context(tc.tile_pool(name="xtp", bufs=3 * KT))
        pproj = ctx.enter_context(
            tc.tile_pool(name="pproj", bufs=2, space="PSUM"))
        psc = ctx.enter_context(tc.tile_pool(name="psc", bufs=2, space="PSUM"))
        sm = ctx.enter_context(tc.tile_pool(name="sm", bufs=3))
        mbp = ctx.enter_context(tc.tile_pool(name="mbp", bufs=2))

        # DMA order matters for the prologue: W/bias/eye (what the
        # projection units need) land first so the first unit starts
        # ~6us in; the U matrices (not needed until the scan) queue
        # after them.
        eye_sb = upool.tile([P, P], BF16, tag="eye", name="eye")
        nc.sync.dma_start(eye_sb[:], eye_d[:])
        u_sb = {}
        w_sb = {}
        b_sb = {}
        for g in "zrh":
            w_sb[g] = wpool.tile([P, KT * D], BF16, tag=f"w{g}", name=f"w{g}")
            nc.sync.dma_start(w_sb[g][:], w_lay[g][:])
            b_sb[g] = bp.tile([P, MT], FP32, tag=f"b{g}", name=f"b{g}")
            nc.sync.dma_start(b_sb[g][:], b4[g][:])

        # SBUF-resident pre-activations: [p, gate, m-tile, t*BL+b]
        xall = xap.tile([P, 3, KT, T_ * BL], BF16, tag="xall", name="xall")

        gate_i = {"z": 0, "r": 1, "h": 2}
        xt_tiles = {}

        def emit_xt_dma(c, kk):
            xt = xtp.tile([P, pcw], BF16, tag="xt", name=f"xt{c}_{kk}")
            nc.sync.dma_start(
                xt[:], xT[kk * P:(kk + 1) * P, c * pcw:(c + 1) * pcw])
            xt_tiles.setdefault(c, {})[kk] = xt

        def emit_xt_dmas(c):
            for kk in range(KT):
                emit_xt_dma(c, kk)

        proj_pending = {}

        def emit_proj_half(c, g, m, half, anchor=None, anchor_dve=None,
                           act_evac=False):
            # one projection unit = 4 k-matmuls + 1 evac; emitted in two
            # halves (2 MMs each) so the per-step PE injection stays
            # small enough to hide in the scan's idle windows.  (The
            # anchor params are unused in the final schedule: both
            # sync=False and sync=True anchors measurably degraded the
            # global schedule, so the Tile scheduler's greedy backfill
            # placement is accepted as-is.)
            key = (c, g, m)
            if half == 0:
                ps = pproj.tile([P, pcw], FP32, tag="pp", name=f"pp{c}{g}{m}")
                proj_pending[key] = ps
            ps = proj_pending[key]
            for kk in ((0, 1) if half == 0 else (2, 3)):
                mm = nc.tensor.matmul(
                    ps[:],
                    w_sb[g][:, kk * D + m * P: kk * D + (m + 1) * P],
                    xt_tiles[c][kk][:],
                    start=(kk == 0), stop=(kk == KT - 1))
                if anchor is not None and kk in (0, 2):
                    add_dep_helper(mm.ins, anchor, sync=True,
                                   reason="proj placement anchor")
            if half == 1:
                del proj_pending[key]
                if act_evac:
                    # prologue: ScalarE is idle there
                    return nc.scalar.activation(
                        xall[:, gate_i[g], m, c * pcw:(c + 1) * pcw],
                        ps[:], AF.Identity, bias=b_sb[g][:, m:m + 1])
                # in-scan: evacuate on DVE (bias add + bf16 cast) to keep
                # the evac off the ScalarE FIFO, where it would delay the
                # next step's sigmoid (GpSimd cannot read PSUM)
                ev = nc.vector.tensor_scalar_add(
                    xall[:, gate_i[g], m, c * pcw:(c + 1) * pcw], ps[:],
                    b_sb[g][:, m:m + 1])
                if anchor_dve is not None:
                    add_dep_helper(ev.ins, anchor_dve, sync=True,
                                   reason="proj evac placement anchor")
                return ev
            return None

        def emit_proj_unit(c, g, m):
            emit_proj_half(c, g, m, 0, act_evac=True)
            return emit_proj_half(c, g, m, 1, act_evac=True)

        # prologue: chunk 0 runs dense before the scan; chunks 1-2
        # interleave into scan chunk 0, chunk c+2 into scan chunk c after
        n_pro = min(sch, 1)
        emit_xt_dmas(0)
        # U matrices are only needed once the scan starts - queue their
        # DMAs behind chunk 0's xT so the prologue projections begin
        # as soon as W/x land
        for g in ("z", "r", "h", "rn"):
            u_sb[g] = upool.tile([P, KT * D], BF16, tag=f"u{g}", name=f"u{g}")
            nc.sync.dma_start(u_sb[g][:], u_lay[g][:])
        for c in range(1, min(sch, 3)):
            emit_xt_dmas(c)
        for c in range(1, min(sch, 3)):
            emit_xt_dmas(c)
        prologue_evacs = []
        for c in range(n_pro):
            for g in "zrh":
                for m in range(MT):
                    prologue_evacs.append(emit_proj_unit(c, g, m).ins)
        half_q = [(c, g, m, half) for c in range(n_pro, sch)
                  for g in "zrh" for m in range(MT) for half in (0, 1)]

        # Each gate is computed as TWO independent PSUM accumulation
        # groups (output m-halves m01 / m23), each in its OWN psum tile.
        # A PSUM reader waits for its accumulation group's STOP matmul,
        # so per-half groups let sigmoid/tanh on the first half fire 8
        # MMs earlier than a single 17-MM group would allow.  Within a
        # half, k01 contraction members run first (they only need the
        # first half of the streamed state, which the blend produces
        # early), k23 members last.
        def gate_half(tag, g, rhs, xv_g, hf, barrier=None, after=None):
            ms = (2 * hf, 2 * hf + 1)
            ps = psc.tile([P, 2, BL], FP32, tag=tag, bufs=1,
                          name=f"ps_{tag}")
            idmm = nc.tensor.matmul(ps[:], eye_sb[:],
                                    xv_g[:, 2 * hf:2 * hf + 2],
                                    start=True, stop=False)
            if barrier:
                # keep the scheduler from dribbling prologue work into the
                # scan: step 0 starts only after the whole prologue
                for e in barrier:
                    add_dep_helper(idmm.ins, e, sync=True,
                                   reason="prologue barrier")
            order = ([(k, m) for k in (0, 1) for m in ms]
                     + [(k, m) for k in (2, 3) for m in ms])
            stop_mm = None
            for i, (kk, m) in enumerate(order):
                mm = nc.tensor.matmul(
                    ps[:, m - 2 * hf],
                    u_sb[g][:, kk * D + m * P: kk * D + (m + 1) * P],
                    rhs[:, kk],
                    start=False,
                    stop=(i == len(order) - 1))
                if False and i == 0 and after is not None:
                    # gate-ordering anchor disabled: testing whether the
                    # scheduler finds a better interleave unconstrained
                    add_dep_helper(mm.ins, after, sync=False,
                                   reason="group ordering")
                stop_mm = mm
            return ps, stop_mm

        h_prev = sm.tile([P, KT, BL], BF16, tag="h", name="h0")
        nc.vector.memset(h_prev[:], 0.0)
        b2_prev = c1n_prev = None

        for t in range(T_):
            c = t // tl
            ti = t % tl
            if ti == 0:
                if 1 <= c <= sch - 3:
                    emit_xt_dmas(c + 2)
                if masked:
                    mb_sb = mbp.tile([P, tl, KT * BL], FP32, tag="m",
                                     name=f"mb{c}")
                    nc.sync.dma_start(
                        mb_sb[:],
                        mb[c * tl:(c + 1) * tl].rearrange("t p x -> p t x"))

            if masked:
                hm = sm.tile([P, KT, BL], BF16, tag="hm")
                nc.vector.tensor_mul(
                    hm[:], h_prev[:],
                    mb_sb[:, ti].rearrange("p (k b) -> p k b", k=KT))
            else:
                hm = h_prev

            xv = xall[:, :, :, t * BL:(t + 1) * BL]

            bar = prologue_evacs if t == 0 else None
            # r gate (two half-groups m01/m23).  On the fast path the
            # streamed state is fed as its two blend summands instead of
            # the materialized h: psum_r = xv + U_r@b2 - U_r@c1n (the
            # negated-weight copy Urn handles the minus).  The c1n
            # members run in the previous step's tanh window (c1n is
            # ready mid-step); only the 4 b2-k23 matmuls remain on the
            # critical h-tail -> sigmoid edge, which removes the final
            # blend subtraction from the serial cycle.
            r_sb = sm.tile([P, KT, BL], BF16, tag="r")
            rhm = sm.tile([P, KT, BL], BF16, tag="rhm")
            ps_r = psc.tile([P, KT, BL], FP32, tag="pr", bufs=1,
                            name="ps_pr")
            idmm = nc.tensor.matmul(ps_r[:], eye_sb[:], xv[:, 1],
                                    start=True, stop=False)
            if bar:
                for e in bar:
                    add_dep_helper(idmm.ins, e, sync=True,
                                   reason="prologue barrier")
            rord = ([(k, m) for k in (0, 1) for m in range(MT)]
                    + [(k, m) for k in (2, 3) for m in range(MT)])
            r_stop = None
            if t == 0 or masked:
                for i, (kk, m) in enumerate(rord):
                    r_stop = nc.tensor.matmul(
                        ps_r[:, m],
                        u_sb["r"][:, kk * D + m * P: kk * D + (m + 1) * P],
                        hm[:, kk],
                        start=False, stop=(i == len(rord) - 1))
            else:
                for kk in range(KT):
                    for m in range(MT):
                        nc.tensor.matmul(
                            ps_r[:, m],
                            u_sb["rn"][:, kk * D + m * P:
                                       kk * D + (m + 1) * P],
                            c1n_prev[:, kk],
                            start=False, stop=False)
                for i, (kk, m) in enumerate(rord):
                    r_stop = nc.tensor.matmul(
                        ps_r[:, m],
                        u_sb["r"][:, kk * D + m * P: kk * D + (m + 1) * P],
                        b2_prev[:, kk],
                        start=False, stop=(i == len(rord) - 1))
            prev_stop = r_stop.ins
            nc.scalar.activation(r_sb[:], ps_r[:], AF.Sigmoid)
            nc.vector.tensor_mul(rhm[:], r_sb[:], hm[:])

            # z gate (complement): zc = 1 - z = sigmoid(-zpre); then the
            # off-critical-path part of the blend in ONE fused DVE op:
            # c1n = (zc - 1) * hm  (so h = zc*hh - c1n).  The z gate is
            # entirely off the critical path, so it stays a SINGLE
            # accumulation group with one sigmoid - a second z ACT would
            # occupy the ScalarE FIFO right where tanh1 needs it.
            zc = sm.tile([P, KT, BL], BF16, tag="zc")
            c1n = sm.tile([P, KT, BL], BF16, tag="c1n")
            ps_z = psc.tile([P, KT, BL], FP32, tag="pz", bufs=1,
                            name="ps_pz")
            zid = nc.tensor.matmul(ps_z[:], eye_sb[:], xv[:, 0],
                                   start=True, stop=False)
            if bar:
                for e in bar:
                    add_dep_helper(zid.ins, e, sync=True,
                                   reason="prologue barrier")
            zord = ([(k, m) for k in (0, 1) for m in range(MT)]
                    + [(k, m) for k in (2, 3) for m in range(MT)])
            for i, (kk, m) in enumerate(zord):
                mm = nc.tensor.matmul(
                    ps_z[:, m],
                    u_sb["z"][:, kk * D + m * P: kk * D + (m + 1) * P],
                    hm[:, kk],
                    start=False, stop=(i == len(zord) - 1))
                if False and i == 0 and prev_stop is not None:
                    add_dep_helper(mm.ins, prev_stop, sync=False,
                                   reason="group ordering")
                prev_stop = mm.ins
            nc.scalar.activation(zc[:], ps_z[:], AF.Sigmoid, scale=-1.0)
            nc.vector.scalar_tensor_tensor(
                c1n[:], zc[:], 1.0, hm[:], ALU.subtract, ALU.mult)

            # h candidate (two half-groups over rhm)
            ps_h = []
            for hf in range(2):
                ps, stop = gate_half(f"ph{hf}", "h", rhm, xv[:, 2], hf,
                                     barrier=bar, after=prev_stop)
                ps_h.append(ps)
                prev_stop = stop.ins

            # critical tail in k-halves: h = zc*hh - c1n; the next step's
            # k0/k1 matmuls only need the first half of h.  Half 1 runs on
            # GpSimd, half 2 on DVE so the two mul+sub chains run in
            # parallel instead of serializing in one FIFO.
            hh = sm.tile([P, KT, BL], BF16, tag="hh")
            b2 = sm.tile([P, KT, BL], BF16, tag="b2")
            h_new = sm.tile([P, KT, BL], BF16, tag="h")
            blend_last = None
            for hf, eng in ((0, nc.gpsimd), (1, nc.vector)):
                sl = slice(2 * hf, 2 * hf + 2)
                nc.scalar.activation(hh[:, sl], ps_h[hf][:], AF.Tanh)
                eng.tensor_mul(b2[:, sl], zc[:, sl], hh[:, sl])
                bl = eng.tensor_sub(h_new[:, sl], b2[:, sl], c1n[:, sl])
                if hf == 1:
                    blend_last = bl.ins
            h_prev = h_new
            b2_prev, c1n_prev = b2, c1n

            # interleave one projection half-unit (2 MMs, ~0.4us) into
            # every other step's tanh/blend tail window, consuming the
            # deadline-ordered queue; chunk c's window hosts any chunk up
            # to c+2 (whose xT DMA has been issued by then).  32 slots
            # per chunk vs 24 halves per chunk keeps every deadline met
            # without overloading any single chunk.
            if half_q and ti % 2 == 0 and half_q[0][0] <= c + 2:
                emit_proj_half(*half_q.pop(0), act_evac=True)

        hout = sm.tile([P, KT, BL], FP32, tag="hout", name="hout")
        nc.vector.tensor_copy(hout[:], h_prev[:])
        for kk in range(KT):
            nc.sync.dma_start(hT_out[kk * P:(kk + 1) * P, :], hout[:, kk])

    nc.compile()
    return nc


_NC_CACHE = {}


def _get_nc(masked):
    if masked not in _NC_CACHE:
        _NC_CACHE[masked] = build_nc(T, masked=masked)
    return _NC_CACHE[masked]


def _bf16(a):
    import ml_dtypes
    return np.asarray(a, dtype=np.float32).astype(ml_dtypes.bfloat16)


def _w_layout(w):
    # [DIN, D] -> [128, KT*D] with lay[p, kk*D + j] = w[kk*128 + p, j]
    return np.ascontiguousarray(
        np.asarray(w, dtype=np.float32).reshape(KT, P, D)
        .transpose(1, 0, 2).reshape(P, KT * D))


def _b_layout(b):
    return np.ascontiguousarray(
        np.asarray(b, dtype=np.float32).reshape(MT, P).T)


def make_in_maps(X, W_z, U_z, b_z, W_r, U_r, b_r, W_h, U_h, b_h, mask,
                 masked):
    X = np.asarray(X, dtype=np.float32)
    shared = {"eye": _bf16(np.eye(P, dtype=np.float32))}
    for g, w, u, b in (("z", W_z, U_z, b_z), ("r", W_r, U_r, b_r),
                       ("h", W_h, U_h, b_h)):
        shared[f"W{g}"] = _bf16(_w_layout(w))
        shared[f"U{g}"] = _bf16(_w_layout(u))
        if g == "r":
            shared["Urn"] = _bf16(_w_layout(-np.asarray(u, np.float32)))
        shared[f"b{g}"] = _b_layout(b)

    in_maps = []
    for c in range(NCORES):
        bsl = slice(c * BL, (c + 1) * BL)
        m = dict(shared)
        m["xT"] = _bf16(np.ascontiguousarray(
            X[bsl].transpose(2, 1, 0).reshape(DIN, T * BL)))
        if masked:
            msh = np.zeros((T, BL), dtype=np.float32)
            msh[1:] = np.asarray(mask)[bsl, :T - 1].T.astype(np.float32)
            m["mb"] = np.ascontiguousarray(
                np.tile(msh[:, None, :], (1, P, KT)))
        in_maps.append(m)
    return in_maps


def kernel(X, W_z, U_z, b_z, W_r, U_r, b_r, W_h, U_h, b_h, mask):
    mask = np.asarray(mask)
    masked = not bool(np.all(mask[:, :T - 1] == 1))
    nc = _get_nc(masked)
    in_maps = make_in_maps(X, W_z, U_z, b_z, W_r, U_r, b_r, W_h, U_h, b_h,
                           mask, masked)
    res = run_bass_kernel_spmd(nc, in_maps, core_ids=list(range(NCORES)))
    out = np.empty((B, D), dtype=np.float32)
    for c in range(NCORES):
        out[c * BL:(c + 1) * BL] = res.results[c]["hT_out"].T
    return out


# revision 39
# speedup vs baseline: 1.0835x; 1.0835x over previous
"""GRU (B=64, T=512, DIN=D=512) on 8 Trainium2 NeuronCores.

Strategy
--------
Data-parallel over batch: each core owns BL = 8 batch rows, weights are
replicated (per the sharding hint).  Per core:

1. Projection phase: xg = X @ W_g + b_g for g in {z, r, h} as bf16 GEMMs
   (X, W pre-converted to bf16 on the host) with W stationary and X^T
   streaming, written into an SBUF-resident pre-activation buffer
   xall[p, g, m, t*BL+b] (bf16) by ScalarE Identity-with-bias ops.
   Chunk 0 runs as a prologue; chunks 1-2 interleave into scan chunk 0,
   chunk c+2 into scan chunk c after that, so projection time is almost
   entirely hidden in the scan's PE idle windows.

2. Scan phase (the sequential part): state is kept transposed,
   hT [128 partitions = d%128, KT=4 k-tiles, BL=8], so that
   - the recurrent matmuls are psum[m] += U[k,m].T @ hmT[k] (U stationary,
     state streaming, output already transposed), and
   - all elementwise work runs on fat [128, *] tiles.
   The x-projection term is accumulated into PSUM by an identity matmul
   (start=True) so the activations read PSUM directly.

   The per-step serial chain is pipelined at m-half / k-half granularity:
   - r-gate MMs are ordered so the m01 half of the r pre-activation
     finishes 8 MMs early; sigmoid(r) and r*hm run in m-halves, letting
     the h-gate's k01 matmuls start while the m23 half is still in the
     ACT/DVE pipe.
   - the h-gate runs in two k-waves (k01 after rhm[m01], k23 after
     rhm[m23]); wave 2 touches the m01 output columns first so tanh on
     the first half starts 4 MMs early.
   - the update gate is computed as zc = sigmoid(-zpre) = 1 - z and the
     blend refactored as h = zc*hh - c1n with c1n = (zc-1)*hm computed
     off the critical path in ONE fused DVE op (scalar_tensor_tensor).
   - blend runs in k-halves so the next step's k01 matmuls start after
     half the blend.

The mask input: reference semantics are h_t = z*(m_{t-1}*h_{t-1}) + ...,
i.e. the *shifted* mask multiplies the previous state.  For the all-ones
mask (what setup_inputs produces) this is the identity, so the fast path
skips the multiply; a general path (host-broadcast shifted mask streamed
from DRAM, one extra DVE mul per step) handles arbitrary 0/1 masks.
"""

import numpy as np
from contextlib import ExitStack

import concourse.bass as bass
import concourse.bacc as bacc
import concourse.mybir as mybir
import concourse.tile as tile
from concourse.tile import add_dep_helper
from concourse.bass_utils import run_bass_kernel_spmd

FP32 = mybir.dt.float32
BF16 = mybir.dt.bfloat16
AF = mybir.ActivationFunctionType
ALU = mybir.AluOpType

B, T, DIN, D = 64, 512, 512, 512
NCORES = 8
BL = B // NCORES            # 8 batch rows per core
KT = DIN // 128             # 4 contraction tiles
MT = D // 128               # 4 output tiles
P = 128


def build_nc(T_=T, masked=False):
    """Build the single-core SPMD program (identical on all 8 cores)."""
    tl = min(64, T_)                     # steps per chunk
    sch = T_ // tl                       # chunks
    pcw = tl * BL                        # chunk width in columns (512)

    nc = bacc.Bacc(None, target_bir_lowering=False, debug=False)

    xT = nc.dram_tensor("xT", [DIN, T_ * BL], BF16, kind="ExternalInput")
    w_lay = {g: nc.dram_tensor(f"W{g}", [P, KT * D], BF16, kind="ExternalInput")
             for g in "zrh"}
    u_lay = {g: nc.dram_tensor(f"U{g}", [P, KT * D], BF16, kind="ExternalInput")
             for g in "zrh"}
    u_lay["rn"] = nc.dram_tensor("Urn", [P, KT * D], BF16,
                                 kind="ExternalInput")
    b4 = {g: nc.dram_tensor(f"b{g}", [P, MT], FP32, kind="ExternalInput")
          for g in "zrh"}
    eye_d = nc.dram_tensor("eye", [P, P], BF16, kind="ExternalInput")
    mb = None
    if masked:
        mb = nc.dram_tensor("mb", [T_, P, KT * BL], FP32, kind="ExternalInput")
    hT_out = nc.dram_tensor("hT_out", [D, BL], FP32, kind="ExternalOutput")

    with tile.TileContext(nc) as tc, ExitStack() as ctx:
        upool = ctx.enter_context(tc.tile_pool(name="upool", bufs=1))
        wpool = ctx.enter_context(tc.tile_pool(name="wpool", bufs=1))
        bp = ctx.enter_context(tc.tile_pool(name="bp", bufs=1))
        xap = ctx.enter_context(tc.tile_pool(name="xap", bufs=1))
        xtp = ctx.enter_context(tc.tile_pool(name="xtp", bufs=3 * KT))
        pproj = ctx.enter_context(
            tc.tile_pool(name="pproj", bufs=2, space="PSUM"))
        psc = ctx.enter_context(tc.tile_pool(name="psc", bufs=2, space="PSUM"))
        sm = ctx.enter_context(tc.tile_pool(name="sm", bufs=3))
        mbp = ctx.enter_context(tc.tile_pool(name="mbp", bufs=2))

        # DMA order matters for the prologue: W/bias/eye (what the
        # projection units need) land first so the first unit starts
        # ~6us in; the U matrices (not needed until the scan) queue
        # after them.
        eye_sb = upool.tile([P, P], BF16, tag="eye", name="eye")
        nc.sync.dma_start(eye_sb[:], eye_d[:])
        u_sb = {}
        w_sb = {}
        b_sb = {}
        for g in "zrh":
            w_sb[g] = wpool.tile([P, KT * D], BF16, tag=f"w{g}", name=f"w{g}")
            nc.sync.dma_start(w_sb[g][:], w_lay[g][:])
            b_sb[g] = bp.tile([P, MT], FP32, tag=f"b{g}", name=f"b{g}")
            nc.sync.dma_start(b_sb[g][:], b4[g][:])

        # SBUF-resident pre-activations: [p, gate, m-tile, t*BL+b]
        xall = xap.tile([P, 3, KT, T_ * BL], BF16, tag="xall", name="xall")

        gate_i = {"z": 0, "r": 1, "h": 2}
        xt_tiles = {}

        def emit_xt_dma(c, kk):
            xt = xtp.tile([P, pcw], BF16, tag="xt", name=f"xt{c}_{kk}")
            nc.sync.dma_start(
                xt[:], xT[kk * P:(kk + 1) * P, c * pcw:(c + 1) * pcw])
            xt_tiles.setdefault(c, {})[kk] = xt

        def emit_xt_dmas(c):
            for kk in range(KT):
                emit_xt_dma(c, kk)

        proj_pending = {}

        def emit_proj_half(c, g, m, half, anchor=None, anchor_dve=None,
                           act_evac=False):
            # one projection unit = 4 k-matmuls + 1 evac; emitted in two
            # halves (2 MMs each) so the per-step PE injection stays
            # small enough to hide in the scan's idle windows.  (The
            # anchor params are unused in the final schedule: both
            # sync=False and sync=True anchors measurably degraded the
            # global schedule, so the Tile scheduler's greedy backfill
            # placement is accepted as-is.)
            key = (c, g, m)
            if half == 0:
                ps = pproj.tile([P, pcw], FP32, tag="pp", name=f"pp{c}{g}{m}")
                proj_pending[key] = ps
            ps = proj_pending[key]
            for kk in ((0, 1) if half == 0 else (2, 3)):
                mm = nc.tensor.matmul(
                    ps[:],
                    w_sb[g][:, kk * D + m * P: kk * D + (m + 1) * P],
                    xt_tiles[c][kk][:],
                    start=(kk == 0), stop=(kk == KT - 1))
                if anchor is not None and kk in (0, 2):
                    add_dep_helper(mm.ins, anchor, sync=True,
                                   reason="proj placement anchor")
            if half == 1:
                del proj_pending[key]
                if act_evac:
                    # prologue: ScalarE is idle there
                    return nc.scalar.activation(
                        xall[:, gate_i[g], m, c * pcw:(c + 1) * pcw],
                        ps[:], AF.Identity, bias=b_sb[g][:, m:m + 1])
                # in-scan: evacuate on DVE (bias add + bf16 cast) to keep
                # the evac off the ScalarE FIFO, where it would delay the
                # next step's sigmoid (GpSimd cannot read PSUM)
                ev = nc.vector.tensor_scalar_add(
                    xall[:, gate_i[g], m, c * pcw:(c + 1) * pcw], ps[:],
                    b_sb[g][:, m:m + 1])
                if anchor_dve is not None:
                    add_dep_helper(ev.ins, anchor_dve, sync=True,
                                   reason="proj evac placement anchor")
                return ev
            return None

        def emit_proj_unit(c, g, m):
            emit_proj_half(c, g, m, 0, act_evac=True)
            return emit_proj_half(c, g, m, 1, act_evac=True)

        # prologue: chunk 0 runs dense before the scan; chunks 1-2
        # interleave into scan chunk 0, chunk c+2 into scan chunk c after
        n_pro = min(sch, 1)
        emit_xt_dmas(0)
        # U matrices are only needed once the scan starts - queue their
        # DMAs behind chunk 0's xT so the prologue projections begin
        # as soon as W/x land
        for g in ("z", "r", "h", "rn"):
            u_sb[g] = upool.tile([P, KT * D], BF16, tag=f"u{g}", name=f"u{g}")
            nc.sync.dma_start(u_sb[g][:], u_lay[g][:])
        for c in range(1, min(sch, 3)):
            emit_xt_dmas(c)
        for c in range(1, min(sch, 3)):
            emit_xt_dmas(c)
        prologue_evacs = []
        for c in range(n_pro):
            for g in "zrh":
                for m in range(MT):
                    prologue_evacs.append(emit_proj_unit(c, g, m).ins)
        half_q = [(c, g, m, half) for c in range(n_pro, sch)
                  for g in "zrh" for m in range(MT) for half in (0, 1)]

        # Each gate is computed as TWO independent PSUM accumulation
        # groups (output m-halves m01 / m23), each in its OWN psum tile.
        # A PSUM reader waits for its accumulation group's STOP matmul,
        # so per-half groups let sigmoid/tanh on the first half fire 8
        # MMs earlier than a single 17-MM group would allow.  Within a
        # half, k01 contraction members run first (they only need the
        # first half of the streamed state, which the blend produces
        # early), k23 members last.
        def gate_half(tag, g, rhs, xv_g, hf, barrier=None, after=None):
            ms = (2 * hf, 2 * hf + 1)
            ps = psc.tile([P, 2, BL], FP32, tag=tag, bufs=1,
                          name=f"ps_{tag}")
            idmm = nc.tensor.matmul(ps[:], eye_sb[:],
                                    xv_g[:, 2 * hf:2 * hf + 2],
                                    start=True, stop=False)
            if barrier:
                # keep the scheduler from dribbling prologue work into the
                # scan: step 0 starts only after the whole prologue
                for e in barrier:
                    add_dep_helper(idmm.ins, e, sync=True,
                                   reason="prologue barrier")
            order = ([(k, m) for k in (0, 1) for m in ms]
                     + [(k, m) for k in (2, 3) for m in ms])
            stop_mm = None
            for i, (kk, m) in enumerate(order):
                mm = nc.tensor.matmul(
                    ps[:, m - 2 * hf],
                    u_sb[g][:, kk * D + m * P: kk * D + (m + 1) * P],
                    rhs[:, kk],
                    start=False,
                    stop=(i == len(order) - 1))
                if i == 0 and after is not None:
                    # keep this group's matmuls from interleaving into the
                    # previous group's block - the previous group's PSUM
                    # completion (which gates an activation on the
                    # critical path) must not be pushed out
                    add_dep_helper(mm.ins, after, sync=False,
                                   reason="group ordering")
                stop_mm = mm
            return ps, stop_mm

        h_prev = sm.tile([P, KT, BL], BF16, tag="h", name="h0")
        nc.vector.memset(h_prev[:], 0.0)
        b2_prev = c1n_prev = None

        for t in range(T_):
            c = t // tl
            ti = t % tl
            if ti == 0:
                if 1 <= c <= sch - 3:
                    emit_xt_dmas(c + 2)
                if masked:
                    mb_sb = mbp.tile([P, tl, KT * BL], FP32, tag="m",
                                     name=f"mb{c}")
                    nc.sync.dma_start(
                        mb_sb[:],
                        mb[c * tl:(c + 1) * tl].rearrange("t p x -> p t x"))

            if masked:
                hm = sm.tile([P, KT, BL], BF16, tag="hm")
                nc.vector.tensor_mul(
                    hm[:], h_prev[:],
                    mb_sb[:, ti].rearrange("p (k b) -> p k b", k=KT))
            else:
                hm = h_prev

            xv = xall[:, :, :, t * BL:(t + 1) * BL]

            bar = prologue_evacs if t == 0 else None
            # r gate (two half-groups m01/m23).  On the fast path the
            # streamed state is fed as its two blend summands instead of
            # the materialized h: psum_r = xv + U_r@b2 - U_r@c1n (the
            # negated-weight copy Urn handles the minus).  The c1n
            # members run in the previous step's tanh window (c1n is
            # ready mid-step); only the 4 b2-k23 matmuls remain on the
            # critical h-tail -> sigmoid edge, which removes the final
            # blend subtraction from the serial cycle.
            r_sb = sm.tile([P, KT, BL], BF16, tag="r")
            rhm = sm.tile([P, KT, BL], BF16, tag="rhm")
            ps_r = psc.tile([P, KT, BL], FP32, tag="pr", bufs=1,
                            name="ps_pr")
            idmm = nc.tensor.matmul(ps_r[:], eye_sb[:], xv[:, 1],
                                    start=True, stop=False)
            if bar:
                for e in bar:
                    add_dep_helper(idmm.ins, e, sync=True,
                                   reason="prologue barrier")
            rord = ([(k, m) for k in (0, 1) for m in range(MT)]
                    + [(k, m) for k in (2, 3) for m in range(MT)])
            r_stop = None
            if t == 0 or masked:
                for i, (kk, m) in enumerate(rord):
                    r_stop = nc.tensor.matmul(
                        ps_r[:, m],
                        u_sb["r"][:, kk * D + m * P: kk * D + (m + 1) * P],
                        hm[:, kk],
                        start=False, stop=(i == len(rord) - 1))
            else:
                for kk in range(KT):
                    for m in range(MT):
                        nc.tensor.matmul(
                            ps_r[:, m],
                            u_sb["rn"][:, kk * D + m * P:
                                       kk * D + (m + 1) * P],
                            c1n_prev[:, kk],
                            start=False, stop=False)
                for i, (kk, m) in enumerate(rord):
                    r_stop = nc.tensor.matmul(
                        ps_r[:, m],
                        u_sb["r"][:, kk * D + m * P: kk * D + (m + 1) * P],
                        b2_prev[:, kk],
                        start=False, stop=(i == len(rord) - 1))
            prev_stop = r_stop.ins
            nc.scalar.activation(r_sb[:], ps_r[:], AF.Sigmoid)
            nc.vector.tensor_mul(rhm[:], r_sb[:], hm[:])

            # z gate (complement): zc = 1 - z = sigmoid(-zpre); then the
            # off-critical-path part of the blend in ONE fused DVE op:
            # c1n = (zc - 1) * hm  (so h = zc*hh - c1n).  The z gate is
            # entirely off the critical path, so it stays a SINGLE
            # accumulation group with one sigmoid - a second z ACT would
            # occupy the ScalarE FIFO right where tanh1 needs it.
            zc = sm.tile([P, KT, BL], BF16, tag="zc")
            c1n = sm.tile([P, KT, BL], BF16, tag="c1n")
            ps_z = psc.tile([P, KT, BL], FP32, tag="pz", bufs=1,
                            name="ps_pz")
            zid = nc.tensor.matmul(ps_z[:], eye_sb[:], xv[:, 0],
                                   start=True, stop=False)
            if bar:
                for e in bar:
                    add_dep_helper(zid.ins, e, sync=True,
                                   reason="prologue barrier")
            zord = ([(k, m) for k in (0, 1) for m in range(MT)]
                    + [(k, m) for k in (2, 3) for m in range(MT)])
            for i, (kk, m) in enumerate(zord):
                mm = nc.tensor.matmul(
                    ps_z[:, m],
                    u_sb["z"][:, kk * D + m * P: kk * D + (m + 1) * P],
                    hm[:, kk],
                    start=False, stop=(i == len(zord) - 1))
                if i == 0 and prev_stop is not None:
                    add_dep_helper(mm.ins, prev_stop, sync=False,
                                   reason="group ordering")
                prev_stop = mm.ins
            nc.scalar.activation(zc[:], ps_z[:], AF.Sigmoid, scale=-1.0)
            nc.vector.scalar_tensor_tensor(
                c1n[:], zc[:], 1.0, hm[:], ALU.subtract, ALU.mult)

            # h candidate (two half-groups over rhm)
            ps_h = []
            for hf in range(2):
                ps, stop = gate_half(f"ph{hf}", "h", rhm, xv[:, 2], hf,
                                     barrier=bar, after=prev_stop)
                ps_h.append(ps)
                prev_stop = stop.ins

            # critical tail in k-halves: h = zc*hh - c1n; the next step's
            # k0/k1 matmuls only need the first half of h.  Half 1 runs on
            # GpSimd, half 2 on DVE so the two mul+sub chains run in
            # parallel instead of serializing in one FIFO.
            hh = sm.tile([P, KT, BL], BF16, tag="hh")
            b2 = sm.tile([P, KT, BL], BF16, tag="b2")
            h_new = sm.tile([P, KT, BL], BF16, tag="h")
            blend_last = None
            for hf, eng in ((0, nc.gpsimd), (1, nc.vector)):
                sl = slice(2 * hf, 2 * hf + 2)
                nc.scalar.activation(hh[:, sl], ps_h[hf][:], AF.Tanh)
                eng.tensor_mul(b2[:, sl], zc[:, sl], hh[:, sl])
                bl = eng.tensor_sub(h_new[:, sl], b2[:, sl], c1n[:, sl])
                if hf == 1:
                    blend_last = bl.ins
            h_prev = h_new
            b2_prev, c1n_prev = b2, c1n

            # interleave one projection half-unit (2 MMs, ~0.4us) into
            # every other step's tanh/blend tail window, consuming the
            # deadline-ordered queue; chunk c's window hosts any chunk up
            # to c+2 (whose xT DMA has been issued by then).  32 slots
            # per chunk vs 24 halves per chunk keeps every deadline met
            # without overloading any single chunk.
            if half_q and ti % 2 == 0 and half_q[0][0] <= c + 2:
                emit_proj_half(*half_q.pop(0), act_evac=True)

        hout = sm.tile([P, KT, BL], FP32, tag="hout", name="hout")
        nc.vector.tensor_copy(hout[:], h_prev[:])
        for kk in range(KT):
            nc.sync.dma_start(hT_out[kk * P:(kk + 1) * P, :], hout[:, kk])

    nc.compile()
    return nc


_NC_CACHE = {}


def _get_nc(masked):
    if masked not in _NC_CACHE:
        _NC_CACHE[masked] = build_nc(T, masked=masked)
    return _NC_CACHE[masked]


def _bf16(a):
    import ml_dtypes
    return np.asarray(a, dtype=np.float32).astype(ml_dtypes.bfloat16)


def _w_layout(w):
    # [DIN, D] -> [128, KT*D] with lay[p, kk*D + j] = w[kk*128 + p, j]
    return np.ascontiguousarray(
        np.asarray(w, dtype=np.float32).reshape(KT, P, D)
        .transpose(1, 0, 2).reshape(P, KT * D))


def _b_layout(b):
    return np.ascontiguousarray(
        np.asarray(b, dtype=np.float32).reshape(MT, P).T)


def make_in_maps(X, W_z, U_z, b_z, W_r, U_r, b_r, W_h, U_h, b_h, mask,
                 masked):
    X = np.asarray(X, dtype=np.float32)
    shared = {"eye": _bf16(np.eye(P, dtype=np.float32))}
    for g, w, u, b in (("z", W_z, U_z, b_z), ("r", W_r, U_r, b_r),
                       ("h", W_h, U_h, b_h)):
        shared[f"W{g}"] = _bf16(_w_layout(w))
        shared[f"U{g}"] = _bf16(_w_layout(u))
        if g == "r":
            shared["Urn"] = _bf16(_w_layout(-np.asarray(u, np.float32)))
        shared[f"b{g}"] = _b_layout(b)

    in_maps = []
    for c in range(NCORES):
        bsl = slice(c * BL, (c + 1) * BL)
        m = dict(shared)
        m["xT"] = _bf16(np.ascontiguousarray(
            X[bsl].transpose(2, 1, 0).reshape(DIN, T * BL)))
        if masked:
            msh = np.zeros((T, BL), dtype=np.float32)
            msh[1:] = np.asarray(mask)[bsl, :T - 1].T.astype(np.float32)
            m["mb"] = np.ascontiguousarray(
                np.tile(msh[:, None, :], (1, P, KT)))
        in_maps.append(m)
    return in_maps


def kernel(X, W_z, U_z, b_z, W_r, U_r, b_r, W_h, U_h, b_h, mask):
    mask = np.asarray(mask)
    masked = not bool(np.all(mask[:, :T - 1] == 1))
    nc = _get_nc(masked)
    in_maps = make_in_maps(X, W_z, U_z, b_z, W_r, U_r, b_r, W_h, U_h, b_h,
                           mask, masked)
    res = run_bass_kernel_spmd(nc, in_maps, core_ids=list(range(NCORES)))
    out = np.empty((B, D), dtype=np.float32)
    for c in range(NCORES):
        out[c * BL:(c + 1) * BL] = res.results[c]["hT_out"].T
    return out
